# revision 9
# baseline (speedup 1.0000x reference)
"""NetVLAD Trainium2 kernel (Bass/Tile), data-parallel over batch on 8 cores.

Math (per batch b):
    x_hat = x / ||x||_2(channel)                    (B, D, H*W), D=512, N=1200
    logits = conv_w @ x_hat                         (K, N), K=64
    a = softmax_K(logits)
    vlad[k,d] = sum_n a[k,n] * x_hat[d,n] - (sum_n a[k,n]) * c[k,d]
    vlad = l2norm_rows(vlad); out = l2norm(flatten(vlad))   # == vlad_rows/8

Device-side structure (v4, fused PE):
  - The v3 per-(n-chunk, d-chunk) transpose + logits matmul pair is FUSED
    into a single 256-column fp32r matmul: stationary = xb chunk, moving =
    vw_a = [ident(128) | wt_a(64) | zeros(64)].  fp32r matmuls with moving
    width >= 256 run at 1 cycle/row (vs 4 below, and 1.5 for is_transpose),
    so one 256-col matmul replaces 192+256 cycles AND one of the two
    LDWEIGHTS+instruction overheads.  Output region [nj, 256] holds the
    transposed x chunk (cols 0:128) and this d-chunk's partial logits
    (cols 128:192).
  - The 4 per-d-chunk logit partials no longer PSUM-accumulate (each fused
    dest also carries a distinct x^T part), so a per-chunk DVE tensor_reduce
    sums the strips [nj, 64, 4] -> lg_sb[nj, 64] in SBUF.  The softmax tail
    reads lg_sb instead of a PSUM parity bank.
  - PSUM: fused regions rotate 3-deep ([P, 3, 4, 256] = 6 banks) + vl +
    asum = 8 banks exactly.  Rotation index is the global chunk counter %3.
  - Both readers of a fused region (the x^T eviction and the logits strip
    reduce) live on DVE back-to-back, so the region WAR folds into a single
    per-engine semaphore wait on the matmul 3 chunks later (walrus S3_LW
    allows only one sync wait per Matmult).  Squares therefore all move to
    ACT; the epilog row-scale also moves to ACT via activation(Copy,
    scale=gkn) with a gpsimd-negated gain column.
  - Warm matmuls (dest: junk cols 2:4 of the asum bank) absorb the x DMA
    part semaphores at j=0/2/4 exactly as in v3; three startup warms touch
    the ident / wt / pad column ranges of vw so the first real matmul
    carries only the x DMA wait.
  - Everything else (3-way split x DMA, softmax tail batching, s-column
    asum trick, rsqrt as exp(-0.5*ln), single ACT table set, deferred
    squares so the tail's exp stays near the ACT queue head, previous-batch
    aggregation interleave, gpsimd for tiny SBUF ops + output DMA) is
    unchanged from v3.

  Hard-won HW facts baked into this design: the PE sustains only ~1.2 GHz
  under load (power throttle); fp32r matmuls with moving dim < 256 run at 4
  cycles/row, >= 256 at 1; matmul outputs are capped at 512 PSUM columns
  and must be fp32; DVE 2x/4x 16-bit modes do not engage for accumulate
  ops; free-axis tensor_reduce is DVE-only; per-PE-instruction overhead is
  ~150ns so fewer/wider matmuls win.
"""

import numpy as np

import concourse.bass as bass
import concourse.mybir as mybir
from concourse import bacc
import concourse.tile as tile
from concourse.bass_utils import run_bass_kernel_spmd
from concourse.masks import make_identity
from concourse.tile_rust import add_dep_helper

F32 = mybir.dt.float32
F32R = mybir.dt.float32r
BF16 = mybir.dt.bfloat16
ALU = mybir.AluOpType
ACTF = mybir.ActivationFunctionType

P = 128
BPC = 8            # batches per core
D = 512
N = 1200
K = 64
DCH = D // P       # 4 d-chunks
NCHUNKS = [(j * P, min(P, N - j * P)) for j in range((N + P - 1) // P)]  # 10
NJ = len(NCHUNKS)
DP2 = D + 2        # xt columns: [x^T | s s]
FW = 256           # fused moving width: [ident | wt_a | pad] (fp32r fast path)
N4 = 256           # first-part pixel count of the split x DMA (chunks 0-1)
N8 = 512           # second-part end (chunks 2-3)
LN_EIGHTH = float(np.log(0.125))

# which previous-batch aggregation chunks run after chunk j's fillers
AGG_SCHED = {3: [0, 1, 2], 4: [3, 4], 5: [5, 6], 6: [7, 8], 7: [9]}

# per-chunk engine for the fused-region release pair (xt eviction + logits
# strip copy, back-to-back on ONE engine so the WAR merges to one sem) and
# for the sum-of-squares pass.
REL_ENG = "a a v a v a a v a v".split()
SQ_ENG = "a v a a v a a v a v".split()


def _emit(nc):
    x = nc.dram_tensor("x", (BPC, D, N), F32R, kind="ExternalInput")
    wt = nc.dram_tensor("wt", (D, K), F32R, kind="ExternalInput")
    cent = nc.dram_tensor("cent", (K, D), F32, kind="ExternalInput")
    out = nc.dram_tensor("out", (BPC, K, D), F32, kind="ExternalOutput")

    with tile.TileContext(nc) as tc:
        with (
            tc.tile_pool(name="const", bufs=1) as const,
            tc.tile_pool(name="xnat", bufs=6) as xnat_pool,
            tc.tile_pool(name="xtsb", bufs=2) as xt_pool,
            tc.tile_pool(name="softmax", bufs=2) as sm_pool,
            tc.tile_pool(name="smalls", bufs=2) as smalls,
            tc.tile_pool(name="epilog", bufs=2) as ep_pool,
            tc.tile_pool(name="psum", bufs=1, space="PSUM") as psum,
        ):
            identf = const.tile([P, P], F32)
            make_identity(nc, identf)
            # fused moving operand per d-chunk: [ident | wt_a | zeros]
            vw = const.tile([P, DCH, FW], F32R)
            for a in range(DCH):
                nc.vector.tensor_copy(vw[:, a, 0:P], identf)
            nc.sync.dma_start(
                vw[:, :, P : P + K], wt[:, :].rearrange("(a p) k -> p a k", p=P)
            )
            nc.gpsimd.memset(vw[:, :, P + K : FW].bitcast(F32), 0.0)
            cent_sb = const.tile([K, D], F32)
            nc.sync.dma_start(cent_sb, cent[:, :])
            ln8 = const.tile([K, 1], F32)
            nc.gpsimd.memset(ln8, LN_EIGHTH)
            # never-read junk outputs for square-accumulate passes
            sqj = const.tile([P, D], BF16)
            sqj2 = const.tile([P, D], BF16)
            sqj3 = const.tile([K, D], BF16)

            # Immortal PSUM (8 banks exactly): fused transpose+logits
            # regions (3-deep rotation), vlad, asum (+2 junk cols for the
            # warm matmuls).
            fused = psum.tile([P, 3, DCH, FW], F32)   # 6 banks
            vl = psum.tile([K, D], F32)               # 1 bank
            asum = psum.tile([K, 4], F32)             # 1 bank

            # Startup warms: touch each producer range of vw so the first
            # real fused matmul carries only the x DMA semaphore.
            w0 = nc.tensor.matmul(
                asum[0:2, 2:4], vw[:, 3, 0:2], vw[:, 3, 0:2],
                start=True, stop=True, skip_group_check=True,
            )
            w1 = nc.tensor.matmul(
                asum[0:2, 2:4], vw[:, 3, P : P + 2], vw[:, 3, P : P + 2],
                start=True, stop=True, skip_group_check=True,
            )
            add_dep_helper(w1.ins, w0.ins, sync=False, reason="warm chain")
            w2 = nc.tensor.matmul(
                asum[0:2, 2:4], vw[:, 3, P + K : P + K + 2],
                vw[:, 3, P + K : P + K + 2],
                start=True, stop=True, skip_group_check=True,
            )
            add_dep_helper(w2.ins, w1.ins, sync=False, reason="warm chain")

            state = {"chunk_ctr": 0}

            def tail_pieces(b):
                """Softmax tail of batch b, as per-chunk filler closures."""
                st = state[b]
                ss, xt = st["ss"], st["xt"]

                def t0():  # ACT: sinv = exp(-0.5*ln(ss))
                    lss = smalls.tile([P, NJ], F32, tag="lss")
                    nc.scalar.activation(lss, ss, ACTF.Ln)
                    sinv = smalls.tile([P, NJ], F32, tag="sinv")
                    nc.scalar.activation(sinv, lss, ACTF.Exp, scale=-0.5)
                    st["sinv"] = sinv

                def t1():  # DVE: batched logit strip sum + prescale;
                    #          Pool: s into xt tail cols
                    sinv = st["sinv"]
                    lg = sm_pool.tile([P, NJ, K], F32, tag="lg")
                    nc.vector.tensor_reduce(
                        lg,
                        st["lgst"].rearrange("p j a k -> p j k a"),
                        axis=mybir.AxisListType.X,
                        op=ALU.add,
                    )
                    lgsc = sm_pool.tile([P, NJ, K], BF16, tag="lgsc")
                    nc.vector.tensor_tensor(
                        lgsc,
                        lg,
                        sinv.unsqueeze(-1).to_broadcast((P, NJ, K)),
                        ALU.mult,
                    )
                    st["lgsc"] = lgsc
                    s = smalls.tile([P, NJ], F32, tag="s")
                    nc.gpsimd.tensor_tensor(s, ss, sinv, ALU.mult)
                    nc.gpsimd.tensor_copy(xt[:, :, D], s)
                    nc.gpsimd.tensor_copy(xt[:, :, D + 1], s)

                def t2():  # ACT: one big exp
                    expt = sm_pool.tile([P, NJ, K], BF16, tag="expt")
                    nc.scalar.activation(expt, st["lgsc"], ACTF.Exp)
                    st["expt"] = expt

                def t3():  # DVE: denominators; Pool: comb
                    den = smalls.tile([P, NJ], F32, tag="den")
                    nc.vector.tensor_reduce(
                        den, st["expt"], axis=mybir.AxisListType.X, op=ALU.add
                    )
                    rden = smalls.tile([P, NJ], F32, tag="rden")
                    nc.vector.reciprocal(rden, den)
                    comb = smalls.tile([P, NJ], F32, tag="comb")
                    nc.gpsimd.tensor_tensor(comb, rden, st["sinv"], ALU.mult)
                    st["comb"] = comb

                def t4():  # DVE: a' = expt * comb (bf16)
                    atp = sm_pool.tile([P, NJ, K], BF16, tag="atp")
                    nc.vector.tensor_tensor(
                        atp,
                        st["expt"],
                        st["comb"].unsqueeze(-1).to_broadcast((P, NJ, K)),
                        ALU.mult,
                    )
                    st["atp"] = atp

                return [t0, t1, t2, t3, t4]

            def phase2_pieces(b):
                """Epilog of batch b (vlad normalization), as fillers."""
                st = state[b]

                def p0():  # DVE: negd = asum*c - vlad
                    negd = ep_pool.tile([K, D], F32, tag="negd")
                    nc.vector.scalar_tensor_tensor(
                        out=negd,
                        in0=cent_sb,
                        scalar=asum[:, 0:1],
                        in1=vl[:, :],
                        op0=ALU.mult,
                        op1=ALU.subtract,
                    )
                    st["negd"] = negd

                def p1():  # ACT: row sum of squares
                    ssk = ep_pool.tile([K, 1], F32, tag="ssk")
                    nc.scalar.activation(
                        sqj3[:, :], st["negd"], ACTF.Square, accum_out=ssk
                    )
                    st["ssk"] = ssk

                def p2():  # ACT: gk = (1/8)*rsqrt(ssk); Pool: gkn = -gk
                    lssk = ep_pool.tile([K, 1], F32, tag="lssk")
                    nc.scalar.activation(lssk, st["ssk"], ACTF.Ln)
                    gk = ep_pool.tile([K, 1], F32, tag="gk")
                    nc.scalar.activation(
                        gk, lssk, ACTF.Exp, scale=-0.5, bias=ln8
                    )
                    gkn = ep_pool.tile([K, 1], F32, tag="gkn")
                    nc.gpsimd.tensor_scalar(
                        out=gkn, in0=gk, scalar1=-1.0, scalar2=None,
                        op0=ALU.mult,
                    )
                    st["gkn"] = gkn

                def p3():  # ACT: scale rows; Pool: output DMA
                    ot = ep_pool.tile([K, D], F32, tag="ot")
                    nc.scalar.activation(
                        ot, st["negd"], ACTF.Copy, scale=st["gkn"]
                    )
                    nc.gpsimd.dma_start(out[b, :, :], ot)
                    state.pop(b)

                return [p0, p1, p2, p3]

            def phase1(b, fillers):
                xb = xnat_pool.tile([P, DCH, N], F32R, tag="xb")
                # split the x load by pixel range so the first chunks'
                # matmuls can start after a fraction of the transfer
                nc.sync.dma_start(
                    xb[:, :, 0:N4],
                    x[b, :, 0:N4].rearrange("(a p) n -> p a n", p=P),
                )
                nc.sync.dma_start(
                    xb[:, :, N4:N8],
                    x[b, :, N4:N8].rearrange("(a p) n -> p a n", p=P),
                )
                nc.sync.dma_start(
                    xb[:, :, N8:N],
                    x[b, :, N8:N].rearrange("(a p) n -> p a n", p=P),
                )

                xt = xt_pool.tile([P, NJ, DP2], BF16, tag="xt")
                lgst = xt_pool.tile([P, NJ, DCH, K], F32, tag="lgst")
                ss = smalls.tile([P, NJ], F32, tag="ss")
                nc.gpsimd.memset(ss, 1.0)

                # warm 1: observes only the xb DMA semaphore.
                warm = nc.tensor.matmul(
                    asum[0:2, 2:4], xb[:, 0, 0:2], xb[:, 0, 0:2],
                    start=True, stop=True, skip_group_check=True,
                )
                if "last_pe" in state:
                    add_dep_helper(
                        warm.ins,
                        state["last_pe"].ins,
                        sync=False,
                        reason="pin batch warm after prior PE work",
                    )
                prev_pe = warm
                for j, (n0, nj) in enumerate(NCHUNKS):
                    r = state["chunk_ctr"] % 3
                    state["chunk_ctr"] += 1
                    if j in (2, 4):
                        # warms 3/4: observe only the second/third x DMA
                        # part, so this chunk's first fused matmul carries
                        # only its region WAR sem (S3_LW allows one sync
                        # wait per Matmult).
                        npart = N4 if j == 2 else N8
                        warm3 = nc.tensor.matmul(
                            asum[0:2, 2:4],
                            xb[:, 0, npart : npart + 2],
                            xb[:, 0, npart : npart + 2],
                            start=True, stop=True, skip_group_check=True,
                        )
                        add_dep_helper(
                            warm3.ins,
                            prev_pe.ins,
                            sync=False,
                            reason="pin DMA-part warm after prior PE work",
                        )
                        prev_pe = warm3
                    for a in range(DCH):
                        prev_pe = nc.tensor.matmul(
                            fused[:nj, r, a, :],
                            xb[:, a, n0 : n0 + nj],
                            vw[:, a, :],
                            start=True,
                            stop=True,
                            skip_group_check=True,
                        )
                    # Release pair on ONE engine (alternating per chunk):
                    # evict transposed x (PSUM fp32 -> SBUF bf16), then copy
                    # the raw logit strips to the SBUF staging tile.  Same
                    # engine back-to-back so the region WAR merges into one
                    # semaphore; the strip summation happens batched in the
                    # tail, off the region-release path.
                    xt_dst = xt[:nj, j, 0:D].rearrange("p (a q) -> p a q", a=DCH)
                    if REL_ENG[j] == "a":
                        nc.scalar.copy(xt_dst, fused[:nj, r, :, 0:P])
                        nc.scalar.copy(
                            lgst[:nj, j], fused[:nj, r, :, P : P + K]
                        )
                    else:
                        nc.vector.tensor_copy(xt_dst, fused[:nj, r, :, 0:P])
                        nc.vector.tensor_copy(
                            lgst[:nj, j], fused[:nj, r, :, P : P + K]
                        )

                    def do_square(jq):
                        n0q, njq = NCHUNKS[jq]
                        if SQ_ENG[jq] == "v":
                            nc.vector.scalar_tensor_tensor(
                                out=sqj[:njq],
                                in0=xt[:njq, jq, 0:D],
                                scalar=1.0,
                                in1=xt[:njq, jq, 0:D],
                                op0=ALU.mult,
                                op1=ALU.mult,
                                accum_out=ss[:njq, jq : jq + 1],
                            )
                        else:
                            nc.scalar.activation(
                                sqj2[:njq],
                                xt[:njq, jq, 0:D],
                                ACTF.Square,
                                accum_out=ss[:njq, jq : jq + 1],
                            )

                    # squares of chunks 0-3 are deferred to iterations 6-9:
                    # they feed only the NEXT batch's tail, and the early
                    # iterations must keep the tail's exp near the ACT queue
                    # head.  The last batch's tail is on the drain critical
                    # path, so its squares run immediately instead.
                    if b == BPC - 1:
                        do_square(j)
                    else:
                        if j >= 4:
                            do_square(j)
                        if j >= 6:
                            do_square(j - 6)
                    # deferred tail/epilog pieces of earlier batches (the
                    # phase2 negd read of vl must be emitted before the
                    # first aggregation matmul overwrites vl)
                    for f in fillers.get(j, ()):
                        f()
                    # interleave the previous batch's aggregation matmuls
                    if b > 0:
                        agg_chunks(b - 1, AGG_SCHED.get(j, ()))

                state[b] = {"xt": xt, "ss": ss, "lgst": lgst}

            def agg_chunks(b, js):
                if not js:
                    return
                st = state[b]
                xt, atp = st["xt"], st["atp"]
                for j in js:
                    n0, nj = NCHUNKS[j]
                    nc.tensor.matmul(
                        vl,
                        atp[:nj, j],
                        xt[:nj, j, 0:D],
                        start=(j == 0),
                        stop=(j == NJ - 1),
                    )
                    last = nc.tensor.matmul(
                        asum[:, 0:2],
                        atp[:nj, j],
                        xt[:nj, j, D : D + 2],
                        start=(j == 0),
                        stop=(j == NJ - 1),
                    )
                    if j == NJ - 1:
                        state["last_pe"] = last

            for b in range(BPC):
                fillers = {}
                if b > 0:
                    t = tail_pieces(b - 1)
                    fillers.setdefault(0, []).extend([t[0], t[1]])
                    fillers.setdefault(2, []).extend([t[2], t[3]])
                    fillers.setdefault(3, []).append(t[4])
                if b > 1:
                    p = phase2_pieces(b - 2)
                    # negd must precede the first aggregation matmul (vl
                    # WAR); the rest go late so they don't congest ACT/DVE
                    # while the tail of the previous batch is in flight.
                    fillers.setdefault(2, []).append(p[0])
                    fillers.setdefault(6, []).append(p[1])
                    fillers.setdefault(7, []).append(p[2])
                    fillers.setdefault(8, []).append(p[3])
                phase1(b, fillers)
            # drain: tail of the last batch, epilog of the last two
            for f in tail_pieces(BPC - 1):
                f()
            for f in phase2_pieces(BPC - 2):
                f()
            agg_chunks(BPC - 1, list(range(NJ)))
            for f in phase2_pieces(BPC - 1):
                f()

    return nc


_NC = None


def _patch_act_tables():
    """Force every ScalarE activation onto the one table set that contains
    {copy, square, ln, exp} so the kernel pays a single ACT_TABLE_LOAD
    instead of thrashing between exp_and_others and natural_log."""
    import concourse.bacc as _bacc_mod
    orig = _bacc_mod.get_activation_tables

    def patched(arch):
        tables = dict(orig(arch))
        assert "natural_log_exp_and_others" in tables
        return {
            name: (funcs if name == "natural_log_exp_and_others" else set())
            for name, funcs in tables.items()
        }

    _bacc_mod.get_activation_tables = patched


def _get_nc():
    global _NC
    if _NC is None:
        _patch_act_tables()
        nc = bacc.Bacc("TRN2", target_bir_lowering=False)
        _emit(nc)
        nc.compile()
        _NC = nc
    return _NC


def _make_in_maps(x, conv_w, centroids):
    B = x.shape[0]
    xs = np.ascontiguousarray(x, dtype=np.float32).reshape(B, D, N)
    wt = np.ascontiguousarray(conv_w.T, dtype=np.float32)
    cent = np.ascontiguousarray(centroids, dtype=np.float32)
    in_maps = []
    for c in range(8):
        in_maps.append(
            {
                "x": np.ascontiguousarray(xs[c * BPC : (c + 1) * BPC]),
                "wt": wt,
                "cent": cent,
            }
        )
    return in_maps


def _run(x, conv_w, centroids, trace=False):
    nc = _get_nc()
    res = run_bass_kernel_spmd(
        nc,
        _make_in_maps(x, conv_w, centroids),
        core_ids=list(range(8)),
        trace=trace,
    )
    outs = [r["out"].reshape(BPC, K * D) for r in res.results]
    full = np.concatenate(outs, axis=0)
    return full, res


def kernel(x, conv_w, centroids):
    full, _ = _run(x, conv_w, centroids, trace=False)
    return full


# revision 15
# speedup vs baseline: 1.0133x; 1.0133x over previous
"""NetVLAD Trainium2 kernel (Bass/Tile), data-parallel over batch on 8 cores.

Math (per batch b):
    x_hat = x / ||x||_2(channel)                    (B, D, H*W), D=512, N=1200
    logits = conv_w @ x_hat                         (K, N), K=64
    a = softmax_K(logits)
    vlad[k,d] = sum_n a[k,n] * x_hat[d,n] - (sum_n a[k,n]) * c[k,d]
    vlad = l2norm_rows(vlad); out = l2norm(flatten(vlad))   # == vlad_rows/8

Device-side structure (v4, fused PE):
  - The v3 per-(n-chunk, d-chunk) transpose + logits matmul pair is FUSED
    into a single 256-column fp32r matmul: stationary = xb chunk, moving =
    vw_a = [ident(128) | wt_a(64) | zeros(64)].  fp32r matmuls with moving
    width >= 256 run at 1 cycle/row (vs 4 below, and 1.5 for is_transpose),
    so one 256-col matmul replaces 192+256 cycles AND one of the two
    LDWEIGHTS+instruction overheads.  Output region [nj, 256] holds the
    transposed x chunk (cols 0:128) and this d-chunk's partial logits
    (cols 128:192).
  - The 4 per-d-chunk logit partials no longer PSUM-accumulate (each fused
    dest also carries a distinct x^T part), so a per-chunk DVE tensor_reduce
    sums the strips [nj, 64, 4] -> lg_sb[nj, 64] in SBUF.  The softmax tail
    reads lg_sb instead of a PSUM parity bank.
  - PSUM: fused regions rotate 3-deep ([P, 3, 4, 256] = 6 banks) + vl +
    asum = 8 banks exactly.  Rotation index is the global chunk counter %3.
  - Both readers of a fused region (the x^T eviction and the logits strip
    reduce) live on DVE back-to-back, so the region WAR folds into a single
    per-engine semaphore wait on the matmul 3 chunks later (walrus S3_LW
    allows only one sync wait per Matmult).  Squares therefore all move to
    ACT; the epilog row-scale also moves to ACT via activation(Copy,
    scale=gkn) with a gpsimd-negated gain column.
  - Warm matmuls (dest: junk cols 2:4 of the asum bank) absorb the x DMA
    part semaphores at j=0/2/4 exactly as in v3; three startup warms touch
    the ident / wt / pad column ranges of vw so the first real matmul
    carries only the x DMA wait.
  - Everything else (3-way split x DMA, softmax tail batching, s-column
    asum trick, rsqrt as exp(-0.5*ln), single ACT table set, deferred
    squares so the tail's exp stays near the ACT queue head, previous-batch
    aggregation interleave, gpsimd for tiny SBUF ops + output DMA) is
    unchanged from v3.

  Hard-won HW facts baked into this design: the PE sustains only ~1.2 GHz
  under load (power throttle); fp32r matmuls with moving dim < 256 run at 4
  cycles/row, >= 256 at 1; matmul outputs are capped at 512 PSUM columns
  and must be fp32; DVE 2x/4x 16-bit modes do not engage for accumulate
  ops; free-axis tensor_reduce is DVE-only; per-PE-instruction overhead is
  ~150ns so fewer/wider matmuls win.
"""

import numpy as np

import concourse.bass as bass
import concourse.mybir as mybir
from concourse import bacc
import concourse.tile as tile
from concourse.bass_utils import run_bass_kernel_spmd
from concourse.masks import make_identity
from concourse.tile_rust import add_dep_helper

F32 = mybir.dt.float32
F32R = mybir.dt.float32r
BF16 = mybir.dt.bfloat16
ALU = mybir.AluOpType
ACTF = mybir.ActivationFunctionType

P = 128
BPC = 8            # batches per core
D = 512
N = 1200
K = 64
DCH = D // P       # 4 d-chunks
NCHUNKS = [(j * P, min(P, N - j * P)) for j in range((N + P - 1) // P)]  # 10
NJ = len(NCHUNKS)
DP2 = D + 2        # xt columns: [x^T | s s]
FW = 256           # fused moving width: [ident | wt_a | pad] (fp32r fast path)
N4 = 256           # first-part pixel count of the split x DMA (chunks 0-1)
N8 = 512           # second-part end (chunks 2-3)
LN_EIGHTH = float(np.log(0.125))

# which previous-batch aggregation chunks run after chunk j's fillers
AGG_SCHED = {3: [0, 1], 4: [2, 3], 5: [4, 5], 6: [6, 7], 7: [8], 8: [9]}

# per-chunk engine for the x^T eviction (the logit strip sum is DVE-only:
# for 'v' chunks it follows the evict on DVE so the region WAR merges into
# one semaphore; for 'a' chunks a warm matmul absorbs the strip-sum WAR
# before the region's next writer) and for the sum-of-squares pass.
REL_ENG = "a a v a a v a a v a".split()
SQ_ENG = "a v a a v a a v a a".split()


def _emit(nc):
    x = nc.dram_tensor("x", (BPC, D, N), F32R, kind="ExternalInput")
    wt = nc.dram_tensor("wt", (D, K), F32R, kind="ExternalInput")
    cent = nc.dram_tensor("cent", (K, D), F32, kind="ExternalInput")
    out = nc.dram_tensor("out", (BPC, K, D), F32, kind="ExternalOutput")

    with tile.TileContext(nc) as tc:
        with (
            tc.tile_pool(name="const", bufs=1) as const,
            tc.tile_pool(name="xnat", bufs=6) as xnat_pool,
            tc.tile_pool(name="xtsb", bufs=2) as xt_pool,
            tc.tile_pool(name="softmax", bufs=2) as sm_pool,
            tc.tile_pool(name="smalls", bufs=2) as smalls,
            tc.tile_pool(name="epilog", bufs=2) as ep_pool,
            tc.tile_pool(name="psum", bufs=1, space="PSUM") as psum,
        ):
            identf = const.tile([P, P], F32)
            make_identity(nc, identf)
            # fused moving operand per d-chunk: [ident | wt_a | zeros]
            vw = const.tile([P, DCH, FW], F32R)
            for a in range(DCH):
                nc.vector.tensor_copy(vw[:, a, 0:P], identf)
            nc.sync.dma_start(
                vw[:, :, P : P + K], wt[:, :].rearrange("(a p) k -> p a k", p=P)
            )
            nc.gpsimd.memset(vw[:, :, P + K : FW].bitcast(F32), 0.0)
            cent_sb = const.tile([K, D], F32)
            nc.sync.dma_start(cent_sb, cent[:, :])
            ln8 = const.tile([K, 1], F32)
            nc.gpsimd.memset(ln8, LN_EIGHTH)
            # never-read junk outputs for square-accumulate passes
            sqj = const.tile([P, D], BF16)
            sqj2 = const.tile([P, D], BF16)
            sqj3 = const.tile([K, D], BF16)

            # Immortal PSUM (8 banks exactly): fused transpose+logits
            # regions (3-deep rotation), vlad, asum (+2 junk cols for the
            # warm matmuls).
            fused = psum.tile([P, 3, DCH, FW], F32)   # 6 banks
            vl = psum.tile([K, D], F32)               # 1 bank
            asum = psum.tile([K, 4], F32)             # 1 bank

            # Startup warms: touch each producer range of vw so the first
            # real fused matmul carries only the x DMA semaphore.
            w0 = nc.tensor.matmul(
                asum[0:2, 2:4], vw[:, 3, 0:2], vw[:, 3, 0:2],
                start=True, stop=True, skip_group_check=True,
            )
            w1 = nc.tensor.matmul(
                asum[0:2, 2:4], vw[:, 3, P : P + 2], vw[:, 3, P : P + 2],
                start=True, stop=True, skip_group_check=True,
            )
            add_dep_helper(w1.ins, w0.ins, sync=False, reason="warm chain")
            w2 = nc.tensor.matmul(
                asum[0:2, 2:4], vw[:, 3, P + K : P + K + 2],
                vw[:, 3, P + K : P + K + 2],
                start=True, stop=True, skip_group_check=True,
            )
            add_dep_helper(w2.ins, w1.ins, sync=False, reason="warm chain")

            state = {"chunk_ctr": 0}

            def tail_pieces(b):
                """Softmax tail of batch b, as per-chunk filler closures."""
                st = state[b]
                ss, xt = st["ss"], st["xt"]

                def t0():  # ACT: sinv = exp(-0.5*ln(ss))
                    lss = smalls.tile([P, NJ], F32, tag="lss")
                    nc.scalar.activation(lss, ss, ACTF.Ln)
                    sinv = smalls.tile([P, NJ], F32, tag="sinv")
                    nc.scalar.activation(sinv, lss, ACTF.Exp, scale=-0.5)
                    st["sinv"] = sinv

                def t1():  # DVE: prescale logits; Pool: s into xt tail cols
                    sinv = st["sinv"]
                    lgsc = sm_pool.tile([P, NJ, K], BF16, tag="lgsc")
                    nc.vector.tensor_tensor(
                        lgsc,
                        st["lg"],
                        sinv.unsqueeze(-1).to_broadcast((P, NJ, K)),
                        ALU.mult,
                    )
                    st["lgsc"] = lgsc
                    s = smalls.tile([P, NJ], F32, tag="s")
                    nc.gpsimd.tensor_tensor(s, ss, sinv, ALU.mult)
                    nc.gpsimd.tensor_copy(xt[:, :, D], s)
                    nc.gpsimd.tensor_copy(xt[:, :, D + 1], s)

                def t2():  # ACT: one big exp
                    expt = sm_pool.tile([P, NJ, K], BF16, tag="expt")
                    nc.scalar.activation(expt, st["lgsc"], ACTF.Exp)
                    st["expt"] = expt

                def t3():  # DVE: denominators; Pool: comb
                    den = smalls.tile([P, NJ], F32, tag="den")
                    nc.vector.tensor_reduce(
                        den, st["expt"], axis=mybir.AxisListType.X, op=ALU.add
                    )
                    rden = smalls.tile([P, NJ], F32, tag="rden")
                    nc.vector.reciprocal(rden, den)
                    comb = smalls.tile([P, NJ], F32, tag="comb")
                    nc.gpsimd.tensor_tensor(comb, rden, st["sinv"], ALU.mult)
                    st["comb"] = comb

                def t4():  # DVE: a' = expt * comb (bf16)
                    atp = sm_pool.tile([P, NJ, K], BF16, tag="atp")
                    nc.vector.tensor_tensor(
                        atp,
                        st["expt"],
                        st["comb"].unsqueeze(-1).to_broadcast((P, NJ, K)),
                        ALU.mult,
                    )
                    st["atp"] = atp

                return [t0, t1, t2, t3, t4]

            def phase2_pieces(b):
                """Epilog of batch b (vlad normalization), as fillers."""
                st = state[b]

                def p0():  # DVE: negd = asum*c - vlad
                    negd = ep_pool.tile([K, D], F32, tag="negd")
                    nc.vector.scalar_tensor_tensor(
                        out=negd,
                        in0=cent_sb,
                        scalar=asum[:, 0:1],
                        in1=vl[:, :],
                        op0=ALU.mult,
                        op1=ALU.subtract,
                    )
                    st["negd"] = negd

                def p1():  # ACT: row sum of squares
                    ssk = ep_pool.tile([K, 1], F32, tag="ssk")
                    nc.scalar.activation(
                        sqj3[:, :], st["negd"], ACTF.Square, accum_out=ssk
                    )
                    st["ssk"] = ssk

                def p2():  # ACT: gk = (1/8)*rsqrt(ssk); Pool: gkn = -gk
                    lssk = ep_pool.tile([K, 1], F32, tag="lssk")
                    nc.scalar.activation(lssk, st["ssk"], ACTF.Ln)
                    gk = ep_pool.tile([K, 1], F32, tag="gk")
                    nc.scalar.activation(
                        gk, lssk, ACTF.Exp, scale=-0.5, bias=ln8
                    )
                    gkn = ep_pool.tile([K, 1], F32, tag="gkn")
                    nc.gpsimd.tensor_scalar(
                        out=gkn, in0=gk, scalar1=-1.0, scalar2=None,
                        op0=ALU.mult,
                    )
                    st["gkn"] = gkn

                def p3():  # ACT: scale rows; Pool: output DMA
                    ot = ep_pool.tile([K, D], F32, tag="ot")
                    nc.scalar.activation(
                        ot, st["negd"], ACTF.Copy, scale=st["gkn"]
                    )
                    nc.gpsimd.dma_start(out[b, :, :], ot)
                    state.pop(b)

                return [p0, p1, p2, p3]

            def phase1(b, fillers):
                xb = xnat_pool.tile([P, DCH, N], F32R, tag="xb")
                # split the x load by pixel range so the first chunks'
                # matmuls can start after a fraction of the transfer
                nc.sync.dma_start(
                    xb[:, :, 0:N4],
                    x[b, :, 0:N4].rearrange("(a p) n -> p a n", p=P),
                )
                nc.sync.dma_start(
                    xb[:, :, N4:N8],
                    x[b, :, N4:N8].rearrange("(a p) n -> p a n", p=P),
                )
                nc.sync.dma_start(
                    xb[:, :, N8:N],
                    x[b, :, N8:N].rearrange("(a p) n -> p a n", p=P),
                )

                xt = xt_pool.tile([P, NJ, DP2], BF16, tag="xt")
                lg = sm_pool.tile([P, NJ, K], F32, tag="lg")
                ss = smalls.tile([P, NJ], F32, tag="ss")
                nc.gpsimd.memset(ss, 1.0)

                # warm 1: observes only the xb DMA semaphore.
                warm = nc.tensor.matmul(
                    asum[0:2, 2:4], xb[:, 0, 0:2], xb[:, 0, 0:2],
                    start=True, stop=True, skip_group_check=True,
                )
                if "last_pe" in state:
                    add_dep_helper(
                        warm.ins,
                        state["last_pe"].ins,
                        sync=False,
                        reason="pin batch warm after prior PE work",
                    )
                prev_pe = warm
                for j, (n0, nj) in enumerate(NCHUNKS):
                    r = state["chunk_ctr"] % 3
                    state["chunk_ctr"] += 1
                    # If this region's previous strip sum ran on DVE while
                    # the evict ran on ACT, a warm matmul touching only the
                    # strip columns absorbs the strip-sum WAR so the first
                    # real matmul carries only the evict WAR.
                    if state.pop(("warm_strip", r), False):
                        warms = nc.tensor.matmul(
                            fused[0:2, r, 0, P : P + 2],
                            vw[:, 3, 0:2],
                            vw[:, 3, 0:2],
                            start=True, stop=True, skip_group_check=True,
                        )
                        add_dep_helper(
                            warms.ins, prev_pe.ins, sync=False,
                            reason="pin strip warm after prior PE work",
                        )
                        prev_pe = warms
                    if j in (2, 4):
                        # warms 3/4: observe only the second/third x DMA
                        # part, so this chunk's first fused matmul carries
                        # only its region WAR sem (S3_LW allows one sync
                        # wait per Matmult).
                        npart = N4 if j == 2 else N8
                        warm3 = nc.tensor.matmul(
                            asum[0:2, 2:4],
                            xb[:, 0, npart : npart + 2],
                            xb[:, 0, npart : npart + 2],
                            start=True, stop=True, skip_group_check=True,
                        )
                        add_dep_helper(
                            warm3.ins,
                            prev_pe.ins,
                            sync=False,
                            reason="pin DMA-part warm after prior PE work",
                        )
                        prev_pe = warm3
                    for a in range(DCH):
                        prev_pe = nc.tensor.matmul(
                            fused[:nj, r, a, :],
                            xb[:, a, n0 : n0 + nj],
                            vw[:, a, :],
                            start=True,
                            stop=True,
                            skip_group_check=True,
                        )
                    # Evict transposed x (PSUM fp32 -> SBUF bf16) on the
                    # alternating engine, then sum the logit strips on DVE.
                    xt_dst = xt[:nj, j, 0:D].rearrange("p (a q) -> p a q", a=DCH)
                    if REL_ENG[j] == "a":
                        nc.scalar.copy(xt_dst, fused[:nj, r, :, 0:P])
                        state[("warm_strip", r)] = True
                    else:
                        nc.vector.tensor_copy(xt_dst, fused[:nj, r, :, 0:P])
                    nc.vector.tensor_reduce(
                        lg[:nj, j, :],
                        fused[:nj, r, :, P : P + K].rearrange("p a k -> p k a"),
                        axis=mybir.AxisListType.X,
                        op=ALU.add,
                    )

                    def do_square(jq):
                        n0q, njq = NCHUNKS[jq]
                        if SQ_ENG[jq] == "v":
                            nc.vector.scalar_tensor_tensor(
                                out=sqj[:njq],
                                in0=xt[:njq, jq, 0:D],
                                scalar=1.0,
                                in1=xt[:njq, jq, 0:D],
                                op0=ALU.mult,
                                op1=ALU.mult,
                                accum_out=ss[:njq, jq : jq + 1],
                            )
                        else:
                            nc.scalar.activation(
                                sqj2[:njq],
                                xt[:njq, jq, 0:D],
                                ACTF.Square,
                                accum_out=ss[:njq, jq : jq + 1],
                            )

                    # squares of chunks 0-3 are deferred to iterations 6-9:
                    # they feed only the NEXT batch's tail, and the early
                    # iterations must keep the tail's exp near the ACT queue
                    # head.  The last batch's tail is on the drain critical
                    # path, so its squares run immediately instead.
                    if b == BPC - 1:
                        do_square(j)
                    else:
                        if j >= 4:
                            do_square(j)
                        if j >= 6:
                            do_square(j - 6)
                    # deferred tail/epilog pieces of earlier batches (the
                    # phase2 negd read of vl must be emitted before the
                    # first aggregation matmul overwrites vl)
                    for f in fillers.get(j, ()):
                        f()
                    # interleave the previous batch's aggregation matmuls
                    if b > 0:
                        agg_chunks(b - 1, AGG_SCHED.get(j, ()))

                state[b] = {"xt": xt, "ss": ss, "lg": lg}

            def agg_chunks(b, js):
                if not js:
                    return
                st = state[b]
                xt, atp = st["xt"], st["atp"]
                for j in js:
                    n0, nj = NCHUNKS[j]
                    nc.tensor.matmul(
                        vl,
                        atp[:nj, j],
                        xt[:nj, j, 0:D],
                        start=(j == 0),
                        stop=(j == NJ - 1),
                    )
                    last = nc.tensor.matmul(
                        asum[:, 0:2],
                        atp[:nj, j],
                        xt[:nj, j, D : D + 2],
                        start=(j == 0),
                        stop=(j == NJ - 1),
                    )
                    if j == NJ - 1:
                        state["last_pe"] = last

            for b in range(BPC):
                fillers = {}
                if b > 0:
                    t = tail_pieces(b - 1)
                    fillers.setdefault(0, []).extend([t[0], t[1]])
                    fillers.setdefault(2, []).extend([t[2], t[3]])
                    fillers.setdefault(3, []).append(t[4])
                if b > 1:
                    p = phase2_pieces(b - 2)
                    # negd must precede the first aggregation matmul (vl
                    # WAR); the rest go late so they don't congest ACT/DVE
                    # while the tail of the previous batch is in flight.
                    fillers.setdefault(2, []).append(p[0])
                    fillers.setdefault(6, []).append(p[1])
                    fillers.setdefault(7, []).append(p[2])
                    fillers.setdefault(8, []).append(p[3])
                phase1(b, fillers)
            # drain: tail of the last batch, epilog of the last two
            for f in tail_pieces(BPC - 1):
                f()
            for f in phase2_pieces(BPC - 2):
                f()
            agg_chunks(BPC - 1, list(range(NJ)))
            for f in phase2_pieces(BPC - 1):
                f()

    return nc


_NC = None


def _patch_act_tables():
    """Force every ScalarE activation onto the one table set that contains
    {copy, square, ln, exp} so the kernel pays a single ACT_TABLE_LOAD
    instead of thrashing between exp_and_others and natural_log."""
    import concourse.bacc as _bacc_mod
    orig = _bacc_mod.get_activation_tables

    def patched(arch):
        tables = dict(orig(arch))
        assert "natural_log_exp_and_others" in tables
        return {
            name: (funcs if name == "natural_log_exp_and_others" else set())
            for name, funcs in tables.items()
        }

    _bacc_mod.get_activation_tables = patched


def _get_nc():
    global _NC
    if _NC is None:
        _patch_act_tables()
        nc = bacc.Bacc("TRN2", target_bir_lowering=False)
        _emit(nc)
        nc.compile()
        _NC = nc
    return _NC


def _make_in_maps(x, conv_w, centroids):
    B = x.shape[0]
    xs = np.ascontiguousarray(x, dtype=np.float32).reshape(B, D, N)
    wt = np.ascontiguousarray(conv_w.T, dtype=np.float32)
    cent = np.ascontiguousarray(centroids, dtype=np.float32)
    in_maps = []
    for c in range(8):
        in_maps.append(
            {
                "x": np.ascontiguousarray(xs[c * BPC : (c + 1) * BPC]),
                "wt": wt,
                "cent": cent,
            }
        )
    return in_maps


def _run(x, conv_w, centroids, trace=False):
    nc = _get_nc()
    res = run_bass_kernel_spmd(
        nc,
        _make_in_maps(x, conv_w, centroids),
        core_ids=list(range(8)),
        trace=trace,
    )
    outs = [r["out"].reshape(BPC, K * D) for r in res.results]
    full = np.concatenate(outs, axis=0)
    return full, res


def kernel(x, conv_w, centroids):
    full, _ = _run(x, conv_w, centroids, trace=False)
    return full


# revision 24
# speedup vs baseline: 1.1202x; 1.1055x over previous
"""NetVLAD Trainium2 kernel (Bass/Tile), data-parallel over batch on 8 cores.

Math (per batch b):
    x_hat = x / ||x||_2(channel)                    (B, D, H*W), D=512, N=1200
    logits = conv_w @ x_hat                         (K, N), K=64
    a = softmax_K(logits)
    vlad[k,d] = sum_n a[k,n] * x_hat[d,n] - (sum_n a[k,n]) * c[k,d]
    vlad = l2norm_rows(vlad); out = l2norm(flatten(vlad))   # == vlad_rows/8

Device-side structure (v4, fused PE):
  - The v3 per-(n-chunk, d-chunk) transpose + logits matmul pair is FUSED
    into a single 256-column fp32r matmul: stationary = xb chunk, moving =
    vw_a = [ident(128) | wt_a(64) | zeros(64)].  fp32r matmuls with moving
    width >= 256 run at 1 cycle/row (vs 4 below, and 1.5 for is_transpose),
    so one 256-col matmul replaces 192+256 cycles AND one of the two
    LDWEIGHTS+instruction overheads.  Output region [nj, 256] holds the
    transposed x chunk (cols 0:128) and this d-chunk's partial logits
    (cols 128:192).
  - The 4 per-d-chunk logit partials no longer PSUM-accumulate (each fused
    dest also carries a distinct x^T part), so a per-chunk DVE tensor_reduce
    sums the strips [nj, 64, 4] -> lg_sb[nj, 64] in SBUF.  The softmax tail
    reads lg_sb instead of a PSUM parity bank.
  - PSUM: fused regions rotate 3-deep ([P, 3, 4, 256] = 6 banks) + vl +
    asum = 8 banks exactly.  Rotation index is the global chunk counter %3.
  - Both readers of a fused region (the x^T eviction and the logits strip
    reduce) live on DVE back-to-back, so the region WAR folds into a single
    per-engine semaphore wait on the matmul 3 chunks later (walrus S3_LW
    allows only one sync wait per Matmult).  Squares therefore all move to
    ACT; the epilog row-scale also moves to ACT via activation(Copy,
    scale=gkn) with a gpsimd-negated gain column.
  - Warm matmuls (dest: junk cols 2:4 of the asum bank) absorb the x DMA
    part semaphores at j=0/2/4 exactly as in v3; three startup warms touch
    the ident / wt / pad column ranges of vw so the first real matmul
    carries only the x DMA wait.
  - Everything else (3-way split x DMA, softmax tail batching, s-column
    asum trick, rsqrt as exp(-0.5*ln), single ACT table set, deferred
    squares so the tail's exp stays near the ACT queue head, previous-batch
    aggregation interleave, gpsimd for tiny SBUF ops + output DMA) is
    unchanged from v3.

  Hard-won HW facts baked into this design: the PE sustains only ~1.2 GHz
  under load (power throttle); fp32r matmuls with moving dim < 256 run at 4
  cycles/row, >= 256 at 1; matmul outputs are capped at 512 PSUM columns
  and must be fp32; DVE 2x/4x 16-bit modes do not engage for accumulate
  ops; free-axis tensor_reduce is DVE-only; per-PE-instruction overhead is
  ~150ns so fewer/wider matmuls win.
"""

import numpy as np

import concourse.bass as bass
import concourse.mybir as mybir
from concourse import bacc
import concourse.tile as tile
from concourse.bass_utils import run_bass_kernel_spmd
from concourse.masks import make_identity
from concourse.tile_rust import add_dep_helper

F32 = mybir.dt.float32
F32R = mybir.dt.float32r
BF16 = mybir.dt.bfloat16
ALU = mybir.AluOpType
ACTF = mybir.ActivationFunctionType

P = 128
BPC = 8            # batches per core
D = 512
N = 1200
K = 64
DCH = D // P       # 4 d-chunks
NCHUNKS = [(j * P, min(P, N - j * P)) for j in range((N + P - 1) // P)]  # 10
NJ = len(NCHUNKS)
DP2 = D + 2        # xt columns: [x^T | s s]
FW = 192           # fused moving width: [ident | wt_a] (bf16, 1 cyc/row)
FSTRIDE = 256      # PSUM column stride per fused region (bank-half aligned)
N4 = 256           # first-part pixel count of the split x DMA (chunks 0-1)
N8 = 512           # second-part end (chunks 2-3)
LN_EIGHTH = float(np.log(0.125))

# which previous-batch aggregation chunks run after chunk j's fillers
AGG_SCHED = {3: [0, 1], 4: [2, 3], 5: [4, 5], 6: [6, 7], 7: [8], 8: [9]}

# per-chunk engine for the x^T eviction (the logit strip sum is DVE-only:
# for 'v' chunks it follows the evict on DVE so the region WAR merges into
# one semaphore; for 'a' chunks a warm matmul absorbs the strip-sum WAR
# before the region's next writer).  Squares all run on ACT.
REL_ENG = "v a v v a v v a v a".split()


def _emit(nc):
    x = nc.dram_tensor("x", (BPC, D, N), BF16, kind="ExternalInput")
    wt = nc.dram_tensor("wt", (D, K), BF16, kind="ExternalInput")
    cent = nc.dram_tensor("cent", (K, D), F32, kind="ExternalInput")
    out = nc.dram_tensor("out", (BPC, K, D), F32, kind="ExternalOutput")

    with tile.TileContext(nc) as tc:
        with (
            tc.tile_pool(name="const", bufs=1) as const,
            tc.tile_pool(name="xnat", bufs=6) as xnat_pool,
            tc.tile_pool(name="xtsb", bufs=2) as xt_pool,
            tc.tile_pool(name="softmax", bufs=2) as sm_pool,
            tc.tile_pool(name="smalls", bufs=2) as smalls,
            tc.tile_pool(name="epilog", bufs=2) as ep_pool,
            tc.tile_pool(name="psum", bufs=1, space="PSUM") as psum,
        ):
            identf = const.tile([P, P], F32)
            make_identity(nc, identf)
            # fused moving operand per d-chunk: [ident | wt_a] (bf16)
            vw = const.tile([P, DCH, FW], BF16)
            for a in range(DCH):
                nc.vector.tensor_copy(vw[:, a, 0:P], identf)
            nc.sync.dma_start(
                vw[:, :, P : P + K], wt[:, :].rearrange("(a p) k -> p a k", p=P)
            )
            cent_sb = const.tile([K, D], F32)
            nc.sync.dma_start(cent_sb, cent[:, :])
            ln8 = const.tile([K, 1], F32)
            nc.gpsimd.memset(ln8, LN_EIGHTH)
            # never-read junk outputs for square-accumulate passes
            sqj = const.tile([P, D], BF16)
            sqj2 = const.tile([P, D], BF16)
            sqj3 = const.tile([K, D], BF16)

            # Immortal PSUM (8 banks exactly): fused transpose+logits
            # regions (3-deep rotation, 192 cols used of each 256-col
            # half-bank-aligned slot), vlad, asum (+2 junk cols for the
            # warm matmuls).
            fused = psum.tile([P, 3, DCH, FSTRIDE], F32)  # 6 banks
            vl = psum.tile([K, D], F32)                   # 1 bank
            asum = psum.tile([K, 4], F32)                 # 1 bank

            # Startup warms: touch each producer range of vw so the first
            # real fused matmul carries only the x DMA semaphore.
            w0 = nc.tensor.matmul(
                asum[0:2, 2:4], vw[:, 3, 0:2], vw[:, 3, 0:2],
                start=True, stop=True, skip_group_check=True,
            )
            w1 = nc.tensor.matmul(
                asum[0:2, 2:4], vw[:, 3, P : P + 2], vw[:, 3, P : P + 2],
                start=True, stop=True, skip_group_check=True,
            )
            add_dep_helper(w1.ins, w0.ins, sync=False, reason="warm chain")

            state = {"chunk_ctr": 0}

            def tail_pieces(b):
                """Softmax tail of batch b, as per-chunk filler closures."""
                st = state[b]
                ss, xt = st["ss"], st["xt"]

                def t0():  # ACT: sinv = exp(-0.5*ln(ss))
                    lss = smalls.tile([P, NJ], F32, tag="lss")
                    nc.scalar.activation(lss, ss, ACTF.Ln)
                    sinv = smalls.tile([P, NJ], F32, tag="sinv")
                    nc.scalar.activation(sinv, lss, ACTF.Exp, scale=-0.5)
                    st["sinv"] = sinv

                def t1():  # DVE: prescale logits; Pool: s into xt tail cols
                    sinv = st["sinv"]
                    lgsc = sm_pool.tile([P, NJ, K], BF16, tag="lgsc")
                    nc.vector.tensor_tensor(
                        lgsc,
                        st["lg"],
                        sinv.unsqueeze(-1).to_broadcast((P, NJ, K)),
                        ALU.mult,
                    )
                    st["lgsc"] = lgsc
                    s = smalls.tile([P, NJ], F32, tag="s")
                    nc.gpsimd.tensor_tensor(s, ss, sinv, ALU.mult)
                    nc.gpsimd.tensor_copy(xt[:, :, D], s)
                    nc.gpsimd.tensor_copy(xt[:, :, D + 1], s)

                def t2():  # ACT: one big exp
                    expt = sm_pool.tile([P, NJ, K], BF16, tag="expt")
                    nc.scalar.activation(expt, st["lgsc"], ACTF.Exp)
                    st["expt"] = expt

                def t3():  # DVE: denominators; Pool: comb
                    den = smalls.tile([P, NJ], F32, tag="den")
                    nc.vector.tensor_reduce(
                        den, st["expt"], axis=mybir.AxisListType.X, op=ALU.add
                    )
                    rden = smalls.tile([P, NJ], F32, tag="rden")
                    nc.vector.reciprocal(rden, den)
                    comb = smalls.tile([P, NJ], F32, tag="comb")
                    nc.gpsimd.tensor_tensor(comb, rden, st["sinv"], ALU.mult)
                    st["comb"] = comb

                def t4():  # DVE: a' = expt * comb (bf16)
                    atp = sm_pool.tile([P, NJ, K], BF16, tag="atp")
                    nc.vector.tensor_tensor(
                        atp,
                        st["expt"],
                        st["comb"].unsqueeze(-1).to_broadcast((P, NJ, K)),
                        ALU.mult,
                    )
                    st["atp"] = atp

                return [t0, t1, t2, t3, t4]

            def phase2_pieces(b):
                """Epilog of batch b (vlad normalization), as fillers."""
                st = state[b]

                def p0():  # DVE: negd = asum*c - vlad
                    negd = ep_pool.tile([K, D], F32, tag="negd")
                    nc.vector.scalar_tensor_tensor(
                        out=negd,
                        in0=cent_sb,
                        scalar=asum[:, 0:1],
                        in1=vl[:, :],
                        op0=ALU.mult,
                        op1=ALU.subtract,
                    )
                    st["negd"] = negd

                def p1():  # ACT: row sum of squares
                    ssk = ep_pool.tile([K, 1], F32, tag="ssk")
                    nc.scalar.activation(
                        sqj3[:, :], st["negd"], ACTF.Square, accum_out=ssk
                    )
                    st["ssk"] = ssk

                def p2():  # ACT: gk = (1/8)*rsqrt(ssk); Pool: gkn = -gk
                    lssk = ep_pool.tile([K, 1], F32, tag="lssk")
                    nc.scalar.activation(lssk, st["ssk"], ACTF.Ln)
                    gk = ep_pool.tile([K, 1], F32, tag="gk")
                    nc.scalar.activation(
                        gk, lssk, ACTF.Exp, scale=-0.5, bias=ln8
                    )
                    gkn = ep_pool.tile([K, 1], F32, tag="gkn")
                    nc.gpsimd.tensor_scalar(
                        out=gkn, in0=gk, scalar1=-1.0, scalar2=None,
                        op0=ALU.mult,
                    )
                    st["gkn"] = gkn

                def p3():  # ACT: scale rows; Pool: output DMA
                    ot = ep_pool.tile([K, D], F32, tag="ot")
                    nc.scalar.activation(
                        ot, st["negd"], ACTF.Copy, scale=st["gkn"]
                    )
                    nc.gpsimd.dma_start(out[b, :, :], ot)
                    state.pop(b)

                return [p0, p1, p2, p3]

            def phase1(b, fillers):
                xb = xnat_pool.tile([P, DCH, N], BF16, tag="xb")
                # split the x load by pixel range so the first chunks'
                # matmuls can start after a fraction of the transfer
                nc.sync.dma_start(
                    xb[:, :, 0:N4],
                    x[b, :, 0:N4].rearrange("(a p) n -> p a n", p=P),
                )
                nc.sync.dma_start(
                    xb[:, :, N4:N8],
                    x[b, :, N4:N8].rearrange("(a p) n -> p a n", p=P),
                )
                nc.sync.dma_start(
                    xb[:, :, N8:N],
                    x[b, :, N8:N].rearrange("(a p) n -> p a n", p=P),
                )

                xt = xt_pool.tile([P, NJ, DP2], BF16, tag="xt")
                lg = sm_pool.tile([P, NJ, K], F32, tag="lg")
                ss = smalls.tile([P, NJ], F32, tag="ss")
                nc.gpsimd.memset(ss, 1.0)

                # warm 1: observes only the xb DMA semaphore.
                warm = nc.tensor.matmul(
                    asum[0:2, 2:4], xb[:, 0, 0:2], xb[:, 0, 0:2],
                    start=True, stop=True, skip_group_check=True,
                )
                if "last_pe" in state:
                    add_dep_helper(
                        warm.ins,
                        state["last_pe"].ins,
                        sync=False,
                        reason="pin batch warm after prior PE work",
                    )
                prev_pe = warm
                for j, (n0, nj) in enumerate(NCHUNKS):
                    r = state["chunk_ctr"] % 3
                    state["chunk_ctr"] += 1
                    # If this region's previous strip sum ran on DVE while
                    # the evict ran on ACT, a warm matmul touching only the
                    # strip columns absorbs the strip-sum WAR so the first
                    # real matmul carries only the evict WAR.
                    if state.pop(("warm_strip", r), False):
                        warms = nc.tensor.matmul(
                            fused[0:2, r, 0, P : P + 2],
                            vw[:, 3, 0:2],
                            vw[:, 3, 0:2],
                            start=True, stop=True, skip_group_check=True,
                        )
                        add_dep_helper(
                            warms.ins, prev_pe.ins, sync=False,
                            reason="pin strip warm after prior PE work",
                        )
                        prev_pe = warms
                    if j in (2, 4):
                        # warms 3/4: observe only the second/third x DMA
                        # part, so this chunk's first fused matmul carries
                        # only its region WAR sem (S3_LW allows one sync
                        # wait per Matmult).
                        npart = N4 if j == 2 else N8
                        warm3 = nc.tensor.matmul(
                            asum[0:2, 2:4],
                            xb[:, 0, npart : npart + 2],
                            xb[:, 0, npart : npart + 2],
                            start=True, stop=True, skip_group_check=True,
                        )
                        add_dep_helper(
                            warm3.ins,
                            prev_pe.ins,
                            sync=False,
                            reason="pin DMA-part warm after prior PE work",
                        )
                        prev_pe = warm3
                    for a in range(DCH):
                        prev_pe = nc.tensor.matmul(
                            fused[:nj, r, a, 0:FW],
                            xb[:, a, n0 : n0 + nj],
                            vw[:, a, :],
                            start=True,
                            stop=True,
                            skip_group_check=True,
                        )
                    # Evict transposed x (PSUM fp32 -> SBUF bf16) on the
                    # alternating engine, then sum the logit strips on DVE.
                    xt_dst = xt[:nj, j, 0:D].rearrange("p (a q) -> p a q", a=DCH)
                    if REL_ENG[j] == "a":
                        nc.scalar.copy(xt_dst, fused[:nj, r, :, 0:P])
                        state[("warm_strip", r)] = True
                    else:
                        nc.vector.tensor_copy(xt_dst, fused[:nj, r, :, 0:P])
                    nc.vector.tensor_reduce(
                        lg[:nj, j, :],
                        fused[:nj, r, :, P : P + K].rearrange("p a k -> p k a"),
                        axis=mybir.AxisListType.X,
                        op=ALU.add,
                    )

                    def do_square(jq):
                        n0q, njq = NCHUNKS[jq]
                        nc.scalar.activation(
                            sqj2[:njq],
                            xt[:njq, jq, 0:D],
                            ACTF.Square,
                            accum_out=ss[:njq, jq : jq + 1],
                        )

                    # squares of chunks 0-3 are deferred to iterations 6-9:
                    # they feed only the NEXT batch's tail, and the early
                    # iterations must keep the tail's exp near the ACT queue
                    # head.  The last batch's tail is on the drain critical
                    # path, so its squares run immediately instead.
                    if b == BPC - 1:
                        do_square(j)
                    else:
                        if j >= 4:
                            do_square(j)
                        if j >= 6:
                            do_square(j - 6)
                    # deferred tail/epilog pieces of earlier batches (the
                    # phase2 negd read of vl must be emitted before the
                    # first aggregation matmul overwrites vl)
                    for f in fillers.get(j, ()):
                        f()
                    # interleave the previous batch's aggregation matmuls
                    if b > 0:
                        agg_chunks(b - 1, AGG_SCHED.get(j, ()))

                state[b] = {"xt": xt, "ss": ss, "lg": lg}

            def agg_chunks(b, js):
                if not js:
                    return
                st = state[b]
                xt, atp = st["xt"], st["atp"]
                for j in js:
                    n0, nj = NCHUNKS[j]
                    nc.tensor.matmul(
                        vl,
                        atp[:nj, j],
                        xt[:nj, j, 0:D],
                        start=(j == 0),
                        stop=(j == NJ - 1),
                    )
                    last = nc.tensor.matmul(
                        asum[:, 0:2],
                        atp[:nj, j],
                        xt[:nj, j, D : D + 2],
                        start=(j == 0),
                        stop=(j == NJ - 1),
                    )
                    if j == NJ - 1:
                        state["last_pe"] = last

            for b in range(BPC):
                fillers = {}
                if b > 0:
                    t = tail_pieces(b - 1)
                    fillers.setdefault(0, []).extend([t[0], t[1]])
                    fillers.setdefault(2, []).extend([t[2], t[3]])
                    fillers.setdefault(3, []).append(t[4])
                if b > 1:
                    p = phase2_pieces(b - 2)
                    # negd must precede the first aggregation matmul (vl
                    # WAR); the rest go late so they don't congest ACT/DVE
                    # while the tail of the previous batch is in flight.
                    fillers.setdefault(2, []).append(p[0])
                    fillers.setdefault(6, []).append(p[1])
                    fillers.setdefault(7, []).append(p[2])
                    fillers.setdefault(8, []).append(p[3])
                phase1(b, fillers)
            # drain: tail of the last batch, epilog of the last two
            for f in tail_pieces(BPC - 1):
                f()
            for f in phase2_pieces(BPC - 2):
                f()
            agg_chunks(BPC - 1, list(range(NJ)))
            for f in phase2_pieces(BPC - 1):
                f()

    return nc


_NC = None


def _patch_act_tables():
    """Force every ScalarE activation onto the one table set that contains
    {copy, square, ln, exp} so the kernel pays a single ACT_TABLE_LOAD
    instead of thrashing between exp_and_others and natural_log."""
    import concourse.bacc as _bacc_mod
    orig = _bacc_mod.get_activation_tables

    def patched(arch):
        tables = dict(orig(arch))
        assert "natural_log_exp_and_others" in tables
        return {
            name: (funcs if name == "natural_log_exp_and_others" else set())
            for name, funcs in tables.items()
        }

    _bacc_mod.get_activation_tables = patched


def _get_nc():
    global _NC
    if _NC is None:
        _patch_act_tables()
        nc = bacc.Bacc("TRN2", target_bir_lowering=False)
        _emit(nc)
        nc.compile()
        _NC = nc
    return _NC


def _make_in_maps(x, conv_w, centroids):
    import ml_dtypes

    bf16 = ml_dtypes.bfloat16
    B = x.shape[0]
    xs = np.ascontiguousarray(
        np.asarray(x, dtype=np.float32).reshape(B, D, N).astype(bf16)
    )
    wt = np.ascontiguousarray(np.asarray(conv_w.T, dtype=np.float32).astype(bf16))
    cent = np.ascontiguousarray(centroids, dtype=np.float32)
    in_maps = []
    for c in range(8):
        in_maps.append(
            {
                "x": np.ascontiguousarray(xs[c * BPC : (c + 1) * BPC]),
                "wt": wt,
                "cent": cent,
            }
        )
    return in_maps


def _run(x, conv_w, centroids, trace=False):
    nc = _get_nc()
    res = run_bass_kernel_spmd(
        nc,
        _make_in_maps(x, conv_w, centroids),
        core_ids=list(range(8)),
        trace=trace,
    )
    outs = [r["out"].reshape(BPC, K * D) for r in res.results]
    full = np.concatenate(outs, axis=0)
    return full, res


def kernel(x, conv_w, centroids):
    full, _ = _run(x, conv_w, centroids, trace=False)
    return full


# revision 29
# speedup vs baseline: 1.3469x; 1.2024x over previous
"""NetVLAD Trainium2 kernel (Bass/Tile), data-parallel over batch on 8 cores.

Math (per batch b):
    x_hat = x / ||x||_2(channel)                    (B, D, H*W), D=512, N=1200
    logits = conv_w @ x_hat                         (K, N), K=64
    a = softmax_K(logits)
    vlad[k,d] = sum_n a[k,n] * x_hat[d,n] - (sum_n a[k,n]) * c[k,d]
    vlad = l2norm_rows(vlad); out = l2norm(flatten(vlad))   # == vlad_rows/8

Device-side structure (v8, DMA-transpose):
  - x is staged host-side as bf16 padded to N=1280 and DMA'd twice per
    batch: once in natural d-major layout (3 n-range parts) for the logits
    matmuls, and once through the DMA TRANSPOSE XBAR (16x128 tiles) into
    xt[p, j, d] = x[d, 10p+j].  This removes every PE transpose and every
    per-chunk PSUM eviction of the old design.  Pad pixels (n >= 1200)
    live in partitions 120:128 of every chunk and are zero.
  - logits are computed k-major: lgT[64, n] = sum_d wt[d,k] x[d,n], with
    wt chunks stationary and 512-wide bf16 moving x slices (1 cyc/row),
    accumulating over the 4 d-chunks into PSUM [64, 1200].  One ACT copy
    evicts lgT to fp16, and a second (SBUF->SBUF) DMA transpose turns it
    into n-major lgn[p, j, k] with the same 10p+j pixel mapping, ready for
    the batched n-major softmax tail.
  - softmax tail unchanged in spirit: sinv = exp(-0.5 ln(ss)); lgsc =
    lgn * sinv; exp; den; arden = expt*rden; atp = arden*sinv (bf16).
  - aggregation: vl[k,d] += atp_j^T @ xt_j over 10 chunks (bf16, 512-wide
    moving).  asum comes from s1[p,k] = sum_j arden (DVE reduce over the
    real partitions) + a tiny ones-moving matmul reducing partitions.
  - ss: 10 Square/STT accum passes over xt[0:120] (the real pixels),
    split ACT/DVE; ss is memset to 1.0 so pad lanes stay finite.
  - PSUM: lgT [64, 2, 1536] (2 parities x 3 bank-aligned 512-col matmul
    dests) + vl + asum = 8 banks.  The only PSUM recycling is the lgT
    parity, reused every other batch - no per-chunk rotation, no
    starvation coupling.
  - Warm matmuls (dest: junk cols of the asum bank) absorb the x DMA part
    semaphores so each first range matmul carries only the lgT parity WAR
    (walrus S3_LW allows one sync wait per Matmult).
  - rsqrt as exp(-0.5*ln), single ACT table set, gpsimd for tiny ops and
    the output DMA, software pipeline: tail of b-1 and epilog of b-2 run
    interleaved with batch b's matmuls.
"""

import numpy as np

import concourse.bass as bass
import concourse.mybir as mybir
from concourse import bacc
import concourse.tile as tile
from concourse.bass_utils import run_bass_kernel_spmd
from concourse.tile_rust import add_dep_helper

F32 = mybir.dt.float32
F16 = mybir.dt.float16
BF16 = mybir.dt.bfloat16
ALU = mybir.AluOpType
ACTF = mybir.ActivationFunctionType

P = 128
BPC = 8            # batches per core
D = 512
N = 1200
NP = 1280          # padded pixel count (XBAR needs free % 128 == 0)
K = 64
DCH = D // P       # 4 d-chunks
NJ = 10            # pixel chunks; xt[p, j, :] = x[:, 128j + p]
NJREAL = [P] * 9 + [48]   # real partitions per chunk (n < 1200)
NRANGES = [(0, 512), (512, 1024), (1024, 1200)]
LN_EIGHTH = float(np.log(0.125))

SQ_ENG = "a v a v a v a v a v".split()


def _emit(nc):
    x = nc.dram_tensor("x", (BPC, D, NP), BF16, kind="ExternalInput")
    wt = nc.dram_tensor("wt", (D, K), BF16, kind="ExternalInput")
    cent = nc.dram_tensor("cent", (K, D), F32, kind="ExternalInput")
    out = nc.dram_tensor("out", (BPC, K, D), F32, kind="ExternalOutput")

    with tile.TileContext(nc) as tc:
        with (
            tc.tile_pool(name="const", bufs=1) as const,
            tc.tile_pool(name="xnat", bufs=3) as xnat_pool,
            tc.tile_pool(name="xtsb", bufs=2) as xt_pool,
            tc.tile_pool(name="softmax", bufs=2) as sm_pool,
            tc.tile_pool(name="smalls", bufs=2) as smalls,
            tc.tile_pool(name="epilog", bufs=2) as ep_pool,
            tc.tile_pool(name="psum", bufs=1, space="PSUM") as psum,
        ):
            wt_sb = const.tile([P, DCH, K], BF16)
            nc.sync.dma_start(wt_sb, wt[:, :].rearrange("(a p) k -> p a k", p=P))
            cent_sb = const.tile([K, D], F32)
            nc.sync.dma_start(cent_sb, cent[:, :])
            ln8 = const.tile([K, 1], F32)
            nc.gpsimd.memset(ln8, LN_EIGHTH)
            onesf = const.tile([P, 2], F32)
            nc.gpsimd.memset(onesf, 1.0)
            # never-read junk outputs for square-accumulate passes
            sqj = const.tile([P, D], BF16)
            sqj2 = const.tile([P, D], BF16)
            sqj3 = const.tile([K, D], BF16)

            # PSUM (8 banks): k-major logits, two parities of 3 bank-aligned
            # 512-col matmul dests each; vlad; asum (+junk cols for warms).
            lgT = psum.tile([K, 2, 3, 512], F32)   # 6 banks
            vl = psum.tile([K, D], F32)            # 1 bank
            asum = psum.tile([K, 4], F32)          # 1 bank

            # Startup warms: absorb the wt DMA and onesf memset semaphores.
            w0 = nc.tensor.matmul(
                asum[0:2, 2:4], wt_sb[:, 3, 0:2], wt_sb[:, 3, 0:2],
                start=True, stop=True, skip_group_check=True,
            )
            w1 = nc.tensor.matmul(
                asum[0:2, 2:4], onesf[:, 0:2], onesf[:, 0:2],
                start=True, stop=True, skip_group_check=True,
            )
            add_dep_helper(w1.ins, w0.ins, sync=False, reason="warm chain")

            state = {}

            def tail_pieces(b):
                """Softmax tail of batch b, as fillers for batch b+1."""
                st = state[b]
                ss = st["ss"]

                def t0():  # ACT: sinv = exp(-0.5*ln(ss))
                    lss = smalls.tile([P, NJ], F32, tag="lss")
                    nc.scalar.activation(lss, ss, ACTF.Ln)
                    sinv = smalls.tile([P, NJ], F32, tag="sinv")
                    nc.scalar.activation(sinv, lss, ACTF.Exp, scale=-0.5)
                    st["sinv"] = sinv

                def t1():  # DVE: prescale logits
                    lgsc = sm_pool.tile([P, NJ, K], BF16, tag="lgsc")
                    nc.vector.tensor_tensor(
                        lgsc,
                        st["lgn"],
                        st["sinv"].unsqueeze(-1).to_broadcast((P, NJ, K)),
                        ALU.mult,
                    )
                    st["lgsc"] = lgsc

                def t2():  # ACT: one big exp
                    expt = sm_pool.tile([P, NJ, K], BF16, tag="expt")
                    nc.scalar.activation(expt, st["lgsc"], ACTF.Exp)
                    st["expt"] = expt

                def t3():  # DVE: denominators
                    den = smalls.tile([P, NJ], F32, tag="den")
                    nc.vector.tensor_reduce(
                        den, st["expt"], axis=mybir.AxisListType.X, op=ALU.add
                    )
                    rden = smalls.tile([P, NJ], F32, tag="rden")
                    nc.vector.reciprocal(rden, den)
                    st["rden"] = rden

                def t4():  # DVE: arden = expt*rden; atp = arden*sinv
                    arden = sm_pool.tile([P, NJ, K], BF16, tag="arden")
                    nc.vector.tensor_tensor(
                        arden,
                        st["expt"],
                        st["rden"].unsqueeze(-1).to_broadcast((P, NJ, K)),
                        ALU.mult,
                    )
                    st["arden"] = arden
                    atp = sm_pool.tile([P, NJ, K], BF16, tag="atp")
                    nc.vector.tensor_tensor(
                        atp,
                        arden,
                        st["sinv"].unsqueeze(-1).to_broadcast((P, NJ, K)),
                        ALU.mult,
                    )
                    st["atp"] = atp

                def t4b():  # DVE: s1[p,k] = sum_j arden over real pixels
                    s1 = smalls.tile([P, K], F32, tag="s1")
                    nc.vector.tensor_reduce(
                        s1,
                        st["arden"][:, 0:9].rearrange("p j k -> p k j"),
                        axis=mybir.AxisListType.X,
                        op=ALU.add,
                    )
                    nc.vector.tensor_tensor(
                        s1[0:48, :], s1[0:48, :], st["arden"][0:48, 9, :],
                        ALU.add,
                    )
                    st["s1"] = s1

                return [t0, t1, t2, t3, t4, t4b]

            def phase2_pieces(b):
                """Epilog of batch b (vlad normalization), as fillers."""
                st = state[b]

                def p0():  # DVE: negd = asum*c - vlad
                    negd = ep_pool.tile([K, D], F32, tag="negd")
                    nc.vector.scalar_tensor_tensor(
                        out=negd,
                        in0=cent_sb,
                        scalar=asum[:, 0:1],
                        in1=vl[:, :],
                        op0=ALU.mult,
                        op1=ALU.subtract,
                    )
                    st["negd"] = negd

                def p1():  # ACT: row sum of squares
                    ssk = ep_pool.tile([K, 1], F32, tag="ssk")
                    nc.scalar.activation(
                        sqj3[:, :], st["negd"], ACTF.Square, accum_out=ssk
                    )
                    st["ssk"] = ssk

                def p2():  # ACT: gk = (1/8)*rsqrt(ssk); Pool: gkn = -gk
                    lssk = ep_pool.tile([K, 1], F32, tag="lssk")
                    nc.scalar.activation(lssk, st["ssk"], ACTF.Ln)
                    gk = ep_pool.tile([K, 1], F32, tag="gk")
                    nc.scalar.activation(
                        gk, lssk, ACTF.Exp, scale=-0.5, bias=ln8
                    )
                    gkn = ep_pool.tile([K, 1], F32, tag="gkn")
                    nc.gpsimd.tensor_scalar(
                        out=gkn, in0=gk, scalar1=-1.0, scalar2=None,
                        op0=ALU.mult,
                    )
                    st["gkn"] = gkn

                def p3():  # ACT: ot = -gk * negd; Pool: output DMA
                    ot = ep_pool.tile([K, D], F32, tag="ot")
                    nc.scalar.activation(
                        ot, st["negd"], ACTF.Copy, scale=st["gkn"]
                    )
                    nc.gpsimd.dma_start(out[b, :, :], ot)
                    state.pop(b)

                return [p0, p1, p2, p3]

            def agg_chunks(b, js):
                st = state[b]
                xt, atp = st["xt"], st["atp"]
                for j in js:
                    nc.tensor.matmul(
                        vl,
                        atp[:, j],
                        xt[:, j, :],
                        start=(j == 0),
                        stop=(j == NJ - 1),
                    )

            def asum_mm(b):
                st = state[b]
                last = nc.tensor.matmul(
                    asum[:, 0:2],
                    st["s1"],
                    onesf[:, 0:2],
                    start=True,
                    stop=True,
                )
                state["last_pe"] = last

            def do_square(b, jq):
                st = state[b]
                nr = NJREAL[jq]
                if SQ_ENG[jq] == "v":
                    nc.vector.scalar_tensor_tensor(
                        out=sqj[:nr],
                        in0=st["xt"][:nr, jq, :],
                        scalar=1.0,
                        in1=st["xt"][:nr, jq, :],
                        op0=ALU.mult,
                        op1=ALU.mult,
                        accum_out=st["ss"][:nr, jq : jq + 1],
                    )
                else:
                    nc.scalar.activation(
                        sqj2[:nr],
                        st["xt"][:nr, jq, :],
                        ACTF.Square,
                        accum_out=st["ss"][:nr, jq : jq + 1],
                    )

            def phase1(b, fillers):
                par = b % 2
                xb = xnat_pool.tile([P, DCH, N], BF16, tag="xb")
                xt = xt_pool.tile([P, NJ, D], BF16, tag="xt")
                lgf16 = sm_pool.tile([K, NP], F16, tag="lgf16")
                lgn = xt_pool.tile([P, NJ, K], F16, tag="lgn")
                ss = smalls.tile([P, NJ], F32, tag="ss")
                st = state[b] = {"xt": xt, "lgn": lgn, "ss": ss}

                # natural-layout x parts (d-major), then the transposed copy
                nc.sync.dma_start(
                    xb[:, :, 0:512],
                    x[b, :, 0:512].rearrange("(a p) n -> p a n", p=P),
                )
                nc.sync.dma_start(
                    xb[:, :, 512:1024],
                    x[b, :, 512:1024].rearrange("(a p) n -> p a n", p=P),
                )
                nc.sync.dma_start(
                    xb[:, :, 1024:N],
                    x[b, :, 1024:N].rearrange("(a p) n -> p a n", p=P),
                )
                nc.sync.dma_start(xt, x[b, :, :], transpose=True)
                nc.gpsimd.memset(ss, 1.0)
                nc.gpsimd.memset(lgf16[:, N:NP], 0.0)

                def emit_warm(src):
                    warm = nc.tensor.matmul(
                        asum[0:2, 2:4], src, src,
                        start=True, stop=True, skip_group_check=True,
                    )
                    if "last_pe" in state:
                        add_dep_helper(
                            warm.ins, state["last_pe"].ins, sync=False,
                            reason="pin warm after prior PE work",
                        )
                    state["last_pe"] = warm

                def run(seg):
                    for f in fillers.get(seg, ()):
                        f()

                # seg0: DMA warm for part 0 + early tail fillers
                emit_warm(xb[:, 0, 0:2])
                run(0)
                for rg, (c0, c1) in enumerate(NRANGES):
                    if rg:
                        emit_warm(xb[:, 0, c0 : c0 + 2])
                    for a in range(DCH):
                        last = nc.tensor.matmul(
                            lgT[:, par, rg, 0 : c1 - c0],
                            wt_sb[:, a, :],
                            xb[:, a, c0:c1],
                            start=(a == 0),
                            stop=(a == DCH - 1),
                            skip_group_check=True,
                        )
                    state["last_pe"] = last
                    run(rg + 1)
                    if rg == 2 and b > 0:
                        agg_chunks(b - 1, range(0, 5))
                # seg4: evict logits to fp16 and transpose them n-major
                nc.scalar.copy(
                    lgf16[:, 0:N],
                    lgT[:, par].rearrange("k r c -> k (r c)")[:, 0:N],
                )
                nc.sync.dma_start(lgn, lgf16, transpose=True)
                if b > 0:
                    agg_chunks(b - 1, range(5, NJ))
                    asum_mm(b - 1)
                for jq in range(0, 5):
                    do_square(b, jq)
                run(4)
                for jq in range(5, NJ):
                    do_square(b, jq)
                run(5)

            for b in range(BPC):
                fillers = {}
                if b > 0:
                    t = tail_pieces(b - 1)
                    fillers[0] = [t[0], t[1]]
                    fillers[1] = [t[2], t[3]]
                    fillers[2] = [t[4], t[5]]
                if b > 1:
                    p = phase2_pieces(b - 2)
                    # negd (vl WAR) must precede the first aggregation MM
                    fillers.setdefault(2, []).append(p[0])
                    fillers[4] = [p[1]]
                    fillers[5] = [p[2], p[3]]
                phase1(b, fillers)
            # drain
            for f in tail_pieces(BPC - 1):
                f()
            for f in phase2_pieces(BPC - 2):
                f()
            agg_chunks(BPC - 1, range(NJ))
            asum_mm(BPC - 1)
            for f in phase2_pieces(BPC - 1):
                f()

    return nc


_NC = None


def _patch_act_tables():
    """Force every ScalarE activation onto the one table set that contains
    {copy, square, ln, exp} so the kernel pays a single ACT_TABLE_LOAD
    instead of thrashing between exp_and_others and natural_log."""
    import concourse.bacc as _bacc_mod
    orig = _bacc_mod.get_activation_tables

    def patched(arch):
        tables = dict(orig(arch))
        assert "natural_log_exp_and_others" in tables
        return {
            name: (funcs if name == "natural_log_exp_and_others" else set())
            for name, funcs in tables.items()
        }

    _bacc_mod.get_activation_tables = patched


def _get_nc():
    global _NC
    if _NC is None:
        _patch_act_tables()
        nc = bacc.Bacc("TRN2", target_bir_lowering=False)
        _emit(nc)
        nc.compile()
        _NC = nc
    return _NC


def _make_in_maps(x, conv_w, centroids):
    import ml_dtypes

    bf16 = ml_dtypes.bfloat16
    B = x.shape[0]
    xp = np.zeros((B, D, NP), dtype=bf16)
    xp[:, :, 0:N] = np.asarray(x, dtype=np.float32).reshape(B, D, N).astype(bf16)
    wt = np.ascontiguousarray(np.asarray(conv_w.T, dtype=np.float32).astype(bf16))
    cent = np.ascontiguousarray(centroids, dtype=np.float32)
    in_maps = []
    for c in range(8):
        in_maps.append(
            {
                "x": np.ascontiguousarray(xp[c * BPC : (c + 1) * BPC]),
                "wt": wt,
                "cent": cent,
            }
        )
    return in_maps


def _run(x, conv_w, centroids, trace=False):
    nc = _get_nc()
    res = run_bass_kernel_spmd(
        nc,
        _make_in_maps(x, conv_w, centroids),
        core_ids=list(range(8)),
        trace=trace,
    )
    outs = [r["out"].reshape(BPC, K * D) for r in res.results]
    full = np.concatenate(outs, axis=0)
    return full, res


def kernel(x, conv_w, centroids):
    full, _ = _run(x, conv_w, centroids, trace=False)
    return full


# revision 32
# speedup vs baseline: 1.4539x; 1.0794x over previous
"""NetVLAD Trainium2 kernel (Bass/Tile), data-parallel over batch on 8 cores.

Math (per batch b):
    x_hat = x / ||x||_2(channel)                    (B, D, H*W), D=512, N=1200
    logits = conv_w @ x_hat                         (K, N), K=64
    a = softmax_K(logits)
    vlad[k,d] = sum_n a[k,n] * x_hat[d,n] - (sum_n a[k,n]) * c[k,d]
    vlad = l2norm_rows(vlad); out = l2norm(flatten(vlad))   # == vlad_rows/8

Device-side structure (v8, DMA-transpose):
  - x is staged host-side as bf16 padded to N=1280 and DMA'd twice per
    batch: once in natural d-major layout (3 n-range parts) for the logits
    matmuls, and once through the DMA TRANSPOSE XBAR (16x128 tiles) into
    xt[p, j, d] = x[d, 10p+j].  This removes every PE transpose and every
    per-chunk PSUM eviction of the old design.  Pad pixels (n >= 1200)
    live in partitions 120:128 of every chunk and are zero.
  - logits are computed k-major: lgT[64, n] = sum_d wt[d,k] x[d,n], with
    wt chunks stationary and 512-wide bf16 moving x slices (1 cyc/row),
    accumulating over the 4 d-chunks into PSUM [64, 1200].  One ACT copy
    evicts lgT to fp16, and a second (SBUF->SBUF) DMA transpose turns it
    into n-major lgn[p, j, k] with the same 10p+j pixel mapping, ready for
    the batched n-major softmax tail.
  - softmax tail unchanged in spirit: sinv = exp(-0.5 ln(ss)); lgsc =
    lgn * sinv; exp; den; arden = expt*rden; atp = arden*sinv (bf16).
  - aggregation: vl[k,d] += atp_j^T @ xt_j over 10 chunks (bf16, 512-wide
    moving).  asum comes from s1[p,k] = sum_j arden (DVE reduce over the
    real partitions) + a tiny ones-moving matmul reducing partitions.
  - ss: 10 Square/STT accum passes over xt[0:120] (the real pixels),
    split ACT/DVE; ss is memset to 1.0 so pad lanes stay finite.
  - PSUM: lgT [64, 2, 1536] (2 parities x 3 bank-aligned 512-col matmul
    dests) + vl + asum = 8 banks.  The only PSUM recycling is the lgT
    parity, reused every other batch - no per-chunk rotation, no
    starvation coupling.
  - Warm matmuls (dest: junk cols of the asum bank) absorb the x DMA part
    semaphores so each first range matmul carries only the lgT parity WAR
    (walrus S3_LW allows one sync wait per Matmult).
  - rsqrt as exp(-0.5*ln), single ACT table set, gpsimd for tiny ops and
    the output DMA, software pipeline: tail of b-1 and epilog of b-2 run
    interleaved with batch b's matmuls.
"""

import numpy as np

import concourse.bass as bass
import concourse.mybir as mybir
from concourse import bacc
import concourse.tile as tile
from concourse.bass_utils import run_bass_kernel_spmd
from concourse.tile_rust import add_dep_helper

F32 = mybir.dt.float32
F16 = mybir.dt.float16
BF16 = mybir.dt.bfloat16
ALU = mybir.AluOpType
ACTF = mybir.ActivationFunctionType

P = 128
BPC = 8            # batches per core
D = 512
N = 1200
NP = 1280          # padded pixel count (XBAR needs free % 128 == 0)
K = 64
DCH = D // P       # 4 d-chunks
NJ = 10            # pixel chunks; xt[p, j, :] = x[:, 128j + p]
NJREAL = [P] * 9 + [48]   # real partitions per chunk (n < 1200)
NRANGES = [(0, 512), (512, 1024), (1024, 1200)]
LN_EIGHTH = float(np.log(0.125))

SQ_ENG = "v a v a v a v a v v".split()


def _emit(nc):
    x = nc.dram_tensor("x", (BPC, D, NP), BF16, kind="ExternalInput")
    wt = nc.dram_tensor("wt", (D, K), BF16, kind="ExternalInput")
    cent = nc.dram_tensor("cent", (K, D), F32, kind="ExternalInput")
    out = nc.dram_tensor("out", (BPC, K, D), F32, kind="ExternalOutput")

    with tile.TileContext(nc) as tc:
        with (
            tc.tile_pool(name="const", bufs=1) as const,
            tc.tile_pool(name="xnat", bufs=3) as xnat_pool,
            tc.tile_pool(name="xtsb", bufs=2) as xt_pool,
            tc.tile_pool(name="softmax", bufs=2) as sm_pool,
            tc.tile_pool(name="smalls", bufs=2) as smalls,
            tc.tile_pool(name="epilog", bufs=2) as ep_pool,
            tc.tile_pool(name="psum", bufs=1, space="PSUM") as psum,
        ):
            wt_sb = const.tile([P, DCH, K], BF16)
            nc.sync.dma_start(wt_sb, wt[:, :].rearrange("(a p) k -> p a k", p=P))
            cent_sb = const.tile([K, D], F32)
            nc.sync.dma_start(cent_sb, cent[:, :])
            ln8 = const.tile([K, 1], F32)
            nc.gpsimd.memset(ln8, LN_EIGHTH)
            onesf = const.tile([P, 2], F32)
            nc.gpsimd.memset(onesf, 1.0)
            # never-read junk outputs for square-accumulate passes
            sqj = const.tile([P, D], BF16)
            sqj2 = const.tile([P, D], BF16)
            sqj3 = const.tile([K, D], BF16)

            # PSUM (8 banks): k-major logits, two parities of 3 bank-aligned
            # 512-col matmul dests each; vlad; asum (+junk cols for warms).
            lgT = psum.tile([K, 2, 3, 512], F32)   # 6 banks
            vl = psum.tile([K, D], F32)            # 1 bank
            asum = psum.tile([K, 4], F32)          # 1 bank

            # Startup warms: absorb the wt DMA and onesf memset semaphores.
            w0 = nc.tensor.matmul(
                asum[0:2, 2:4], wt_sb[:, 3, 0:2], wt_sb[:, 3, 0:2],
                start=True, stop=True, skip_group_check=True,
            )
            w1 = nc.tensor.matmul(
                asum[0:2, 2:4], onesf[:, 0:2], onesf[:, 0:2],
                start=True, stop=True, skip_group_check=True,
            )
            add_dep_helper(w1.ins, w0.ins, sync=False, reason="warm chain")

            state = {}

            def tail_pieces(b):
                """Softmax tail of batch b, as fillers for batch b+1."""
                st = state[b]
                ss = st["ss"]

                def t0():  # ACT: sinv = exp(-0.5*ln(ss))
                    lss = smalls.tile([P, NJ], F32, tag="lss")
                    nc.scalar.activation(lss, ss, ACTF.Ln)
                    sinv = smalls.tile([P, NJ], F32, tag="sinv")
                    nc.scalar.activation(sinv, lss, ACTF.Exp, scale=-0.5)
                    st["sinv"] = sinv

                def t1():  # DVE: prescale logits
                    lgsc = sm_pool.tile([P, NJ, K], BF16, tag="lgsc")
                    nc.vector.tensor_tensor(
                        lgsc,
                        st["lgn"],
                        st["sinv"].unsqueeze(-1).to_broadcast((P, NJ, K)),
                        ALU.mult,
                    )
                    st["lgsc"] = lgsc

                def t2():  # ACT: one big exp
                    expt = sm_pool.tile([P, NJ, K], BF16, tag="expt")
                    nc.scalar.activation(expt, st["lgsc"], ACTF.Exp)
                    st["expt"] = expt

                def t3():  # DVE: denominators
                    den = smalls.tile([P, NJ], F32, tag="den")
                    nc.vector.tensor_reduce(
                        den, st["expt"], axis=mybir.AxisListType.X, op=ALU.add
                    )
                    rden = smalls.tile([P, NJ], F32, tag="rden")
                    nc.vector.reciprocal(rden, den)
                    st["rden"] = rden

                def t4():  # DVE: arden = expt*rden; atp = arden*sinv
                    arden = sm_pool.tile([P, NJ, K], BF16, tag="arden")
                    nc.vector.tensor_tensor(
                        arden,
                        st["expt"],
                        st["rden"].unsqueeze(-1).to_broadcast((P, NJ, K)),
                        ALU.mult,
                    )
                    st["arden"] = arden
                    atp = sm_pool.tile([P, NJ, K], BF16, tag="atp")
                    nc.vector.tensor_tensor(
                        atp,
                        arden,
                        st["sinv"].unsqueeze(-1).to_broadcast((P, NJ, K)),
                        ALU.mult,
                    )
                    st["atp"] = atp

                def t4b():  # DVE: s1[p,k] = sum_j arden over real pixels
                    s1 = smalls.tile([P, K], F32, tag="s1")
                    nc.vector.tensor_reduce(
                        s1,
                        st["arden"][:, 0:9].rearrange("p j k -> p k j"),
                        axis=mybir.AxisListType.X,
                        op=ALU.add,
                    )
                    nc.vector.tensor_tensor(
                        s1[0:48, :], s1[0:48, :], st["arden"][0:48, 9, :],
                        ALU.add,
                    )
                    st["s1"] = s1

                return [t0, t1, t2, t3, t4, t4b]

            def phase2_pieces(b):
                """Epilog of batch b (vlad normalization), as fillers."""
                st = state[b]

                def p0():  # DVE: negd = asum*c - vlad
                    negd = ep_pool.tile([K, D], F32, tag="negd")
                    nc.vector.scalar_tensor_tensor(
                        out=negd,
                        in0=cent_sb,
                        scalar=asum[:, 0:1],
                        in1=vl[:, :],
                        op0=ALU.mult,
                        op1=ALU.subtract,
                    )
                    st["negd"] = negd

                def p1():  # ACT: row sum of squares
                    ssk = ep_pool.tile([K, 1], F32, tag="ssk")
                    nc.scalar.activation(
                        sqj3[:, :], st["negd"], ACTF.Square, accum_out=ssk
                    )
                    st["ssk"] = ssk

                def p2():  # ACT: gk = (1/8)*rsqrt(ssk); Pool: gkn = -gk
                    lssk = ep_pool.tile([K, 1], F32, tag="lssk")
                    nc.scalar.activation(lssk, st["ssk"], ACTF.Ln)
                    gk = ep_pool.tile([K, 1], F32, tag="gk")
                    nc.scalar.activation(
                        gk, lssk, ACTF.Exp, scale=-0.5, bias=ln8
                    )
                    gkn = ep_pool.tile([K, 1], F32, tag="gkn")
                    nc.gpsimd.tensor_scalar(
                        out=gkn, in0=gk, scalar1=-1.0, scalar2=None,
                        op0=ALU.mult,
                    )
                    st["gkn"] = gkn

                def p3():  # ACT: ot = -gk * negd; Pool: output DMA
                    ot = ep_pool.tile([K, D], F32, tag="ot")
                    nc.scalar.activation(
                        ot, st["negd"], ACTF.Copy, scale=st["gkn"]
                    )
                    nc.gpsimd.dma_start(out[b, :, :], ot)
                    state.pop(b)

                return [p0, p1, p2, p3]

            def agg_chunks(b, js):
                st = state[b]
                xt, atp = st["xt"], st["atp"]
                for j in js:
                    nc.tensor.matmul(
                        vl,
                        atp[:, j],
                        xt[:, j, :],
                        start=(j == 0),
                        stop=(j == NJ - 1),
                    )

            def asum_mm(b):
                st = state[b]
                last = nc.tensor.matmul(
                    asum[:, 0:2],
                    st["s1"],
                    onesf[:, 0:2],
                    start=True,
                    stop=True,
                )
                state["last_pe"] = last

            def do_square(b, jq):
                st = state[b]
                nr = NJREAL[jq]
                if SQ_ENG[jq] == "v":
                    nc.vector.scalar_tensor_tensor(
                        out=sqj[:nr],
                        in0=st["xt"][:nr, jq, :],
                        scalar=1.0,
                        in1=st["xt"][:nr, jq, :],
                        op0=ALU.mult,
                        op1=ALU.mult,
                        accum_out=st["ss"][:nr, jq : jq + 1],
                    )
                else:
                    nc.scalar.activation(
                        sqj2[:nr],
                        st["xt"][:nr, jq, :],
                        ACTF.Square,
                        accum_out=st["ss"][:nr, jq : jq + 1],
                    )

            def phase1(b, fillers):
                par = b % 2
                xb = xnat_pool.tile([P, DCH, N], BF16, tag="xb")
                xt = xt_pool.tile([P, NJ, D], BF16, tag="xt")
                lgf16 = sm_pool.tile([K, NP], F16, tag="lgf16")
                lgn = xt_pool.tile([P, NJ, K], F16, tag="lgn")
                ss = smalls.tile([P, NJ], F32, tag="ss")
                st = state[b] = {"xt": xt, "lgn": lgn, "ss": ss}

                # natural-layout x parts (d-major), then the transposed copy
                nc.sync.dma_start(
                    xb[:, :, 0:512],
                    x[b, :, 0:512].rearrange("(a p) n -> p a n", p=P),
                )
                nc.sync.dma_start(
                    xb[:, :, 512:1024],
                    x[b, :, 512:1024].rearrange("(a p) n -> p a n", p=P),
                )
                nc.sync.dma_start(
                    xb[:, :, 1024:N],
                    x[b, :, 1024:N].rearrange("(a p) n -> p a n", p=P),
                )
                nc.sync.dma_start(xt, x[b, :, :], transpose=True)
                nc.gpsimd.memset(ss, 1.0)
                nc.gpsimd.memset(lgf16[:, N:NP], 0.0)

                def emit_warm(src):
                    warm = nc.tensor.matmul(
                        asum[0:2, 2:4], src, src,
                        start=True, stop=True, skip_group_check=True,
                    )
                    if "last_pe" in state:
                        add_dep_helper(
                            warm.ins, state["last_pe"].ins, sync=False,
                            reason="pin warm after prior PE work",
                        )
                    state["last_pe"] = warm

                def run(seg):
                    for f in fillers.get(seg, ()):
                        f()

                # seg0: DMA warm for part 0 + early tail fillers
                emit_warm(xb[:, 0, 0:2])
                run(0)
                for rg, (c0, c1) in enumerate(NRANGES):
                    if rg:
                        emit_warm(xb[:, 0, c0 : c0 + 2])
                    for a in range(DCH):
                        last = nc.tensor.matmul(
                            lgT[:, par, rg, 0 : c1 - c0],
                            wt_sb[:, a, :],
                            xb[:, a, c0:c1],
                            start=(a == 0),
                            stop=(a == DCH - 1),
                            skip_group_check=True,
                        )
                    state["last_pe"] = last
                    run(rg + 1)
                    if rg == 2 and b > 0:
                        agg_chunks(b - 1, range(0, 5))
                # seg4: evict logits to fp16 and transpose them n-major
                nc.scalar.copy(
                    lgf16[:, 0:N],
                    lgT[:, par].rearrange("k r c -> k (r c)")[:, 0:N],
                )
                nc.sync.dma_start(lgn, lgf16, transpose=True)
                if b > 0:
                    agg_chunks(b - 1, range(5, NJ))
                    asum_mm(b - 1)
                for jq in range(0, 5):
                    do_square(b, jq)
                run(4)
                for jq in range(5, NJ):
                    do_square(b, jq)
                run(5)

            for b in range(BPC):
                fillers = {}
                if b > 0:
                    t = tail_pieces(b - 1)
                    fillers[0] = [t[0], t[1]]
                    fillers[1] = [t[2], t[3]]
                    fillers[2] = [t[4], t[5]]
                if b > 1:
                    p = phase2_pieces(b - 2)
                    # negd (vl WAR) must precede the first aggregation MM
                    fillers.setdefault(2, []).append(p[0])
                    fillers[4] = [p[1]]
                    fillers[5] = [p[2], p[3]]
                phase1(b, fillers)
            # drain
            for f in tail_pieces(BPC - 1):
                f()
            for f in phase2_pieces(BPC - 2):
                f()
            agg_chunks(BPC - 1, range(NJ))
            asum_mm(BPC - 1)
            for f in phase2_pieces(BPC - 1):
                f()

    return nc


_NC = None


def _patch_act_tables():
    """Force every ScalarE activation onto the one table set that contains
    {copy, square, ln, exp} so the kernel pays a single ACT_TABLE_LOAD
    instead of thrashing between exp_and_others and natural_log."""
    import concourse.bacc as _bacc_mod
    orig = _bacc_mod.get_activation_tables

    def patched(arch):
        tables = dict(orig(arch))
        assert "natural_log_exp_and_others" in tables
        return {
            name: (funcs if name == "natural_log_exp_and_others" else set())
            for name, funcs in tables.items()
        }

    _bacc_mod.get_activation_tables = patched


def _get_nc():
    global _NC
    if _NC is None:
        _patch_act_tables()
        nc = bacc.Bacc("TRN2", target_bir_lowering=False)
        _emit(nc)
        nc.compile()
        _NC = nc
    return _NC


def _make_in_maps(x, conv_w, centroids):
    import ml_dtypes

    bf16 = ml_dtypes.bfloat16
    B = x.shape[0]
    xp = np.zeros((B, D, NP), dtype=bf16)
    xp[:, :, 0:N] = np.asarray(x, dtype=np.float32).reshape(B, D, N).astype(bf16)
    wt = np.ascontiguousarray(np.asarray(conv_w.T, dtype=np.float32).astype(bf16))
    cent = np.ascontiguousarray(centroids, dtype=np.float32)
    in_maps = []
    for c in range(8):
        in_maps.append(
            {
                "x": np.ascontiguousarray(xp[c * BPC : (c + 1) * BPC]),
                "wt": wt,
                "cent": cent,
            }
        )
    return in_maps


def _run(x, conv_w, centroids, trace=False):
    nc = _get_nc()
    res = run_bass_kernel_spmd(
        nc,
        _make_in_maps(x, conv_w, centroids),
        core_ids=list(range(8)),
        trace=trace,
    )
    outs = [r["out"].reshape(BPC, K * D) for r in res.results]
    full = np.concatenate(outs, axis=0)
    return full, res


def kernel(x, conv_w, centroids):
    full, _ = _run(x, conv_w, centroids, trace=False)
    return full
